# revision 3
# baseline (speedup 1.0000x reference)
"""Trainium2 Bass kernel for ConvPool (3x3 VALID conv + bias + relu + 2x2 maxpool).

Full-input contract: kernel(x, weight, bias) -> (32, 64, 3969) float32.
Data-parallel over batch across 8 NeuronCores (4 images per core).

Per-core algorithm:
  - x is loaded into SBUF 3x (one copy per horizontal filter tap m), each
    copy shifted by m columns, at partition (m*16 + c).  Two images live at
    partition bases 0 and 64.
  - Conv = 3 PSUM-accumulating matmuls (one per vertical tap n) with K=48,
    M=64; the n-offset is a pure free-dim offset into the shifted x copy.
    The two images run in disjoint PE quadrants (rows 0-47 x cols 0-63 and
    rows 64-111 x cols 64-127) so their matmuls execute concurrently.
  - bf16 matmul inputs (cast during the SWDGE load DMA): full PE rate,
    fp32 PSUM accumulation; everything after the matmul is fp32.
  - ScalarE evacuates PSUM with relu(conv + bias) in one pass (exact:
    max-pool commutes with the monotone x -> relu(x+b)).
  - Maxpool: two DVE tensor_tensor(max) passes on SBUF (column pairs, then
    row pairs into the output stage), then one 2 MB DMA per image pair.
"""

import numpy as np

import concourse.bass as bass
import concourse.bacc as bacc
import concourse.mybir as mybir
import concourse.tile as tile
from concourse.bass_utils import run_bass_kernel_spmd

N_CORES = 8
B, C, H, W = 32, 16, 128, 128
FD, OUT, POOL = 3, 64, 2
BPC = B // N_CORES            # images per core
HC = H - FD + 1               # conv output height/width = 126
HP = HC // POOL               # pooled height/width = 63
NPIX = HP * HP                # 3969
ROWS_PER_TILE = 4             # conv rows per PSUM tile (N = 4*128 = 512)

f32 = mybir.dt.float32
bf16 = mybir.dt.bfloat16
MAX = mybir.AluOpType.max

_cache: dict = {}


def _build():
    nc = bacc.Bacc("TRN2", target_bir_lowering=False, debug=False)
    x_d = nc.dram_tensor("x", [BPC, C, H, W], f32, kind="ExternalInput").ap()
    w_d = nc.dram_tensor("weight", [C * FD * FD, OUT], f32,
                         kind="ExternalInput").ap()
    b_d = nc.dram_tensor("bias", [OUT], f32, kind="ExternalInput").ap()
    y_d = nc.dram_tensor("y", [BPC, OUT, NPIX], f32, kind="ExternalOutput").ap()

    with tile.TileContext(nc) as tc:
        with (
            tc.tile_pool(name="const", bufs=1) as const,
            tc.tile_pool(name="xrep", bufs=2) as xpool,
            tc.tile_pool(name="psum", bufs=4, space="PSUM") as psum,
            tc.tile_pool(name="rbuf", bufs=3) as rpool,
            tc.tile_pool(name="hbuf", bufs=3) as hpool,
            tc.tile_pool(name="ostage", bufs=2) as opool,
        ):
            # Weights: partition (img*64 + m*16 + c), free (n*64 + o).
            w_sb = const.tile([128, FD * OUT], bf16)
            w_src = w_d.rearrange("(c n m) o -> m c n o", c=C, n=FD, m=FD)
            for img in range(2):
                for m in range(FD):
                    p0 = img * 64 + m * 16
                    dst = w_sb[p0:p0 + C, :].rearrange("p (n o) -> p n o", o=OUT)
                    nc.gpsimd.dma_start(dst, w_src[m])

            # Bias: per-partition scalar, duplicated for both image halves.
            bias_sb = const.tile([128, 1], f32)
            b_src = b_d.rearrange("(o u) -> o u", u=1)
            nc.sync.dma_start(bias_sb[0:OUT, :], b_src)
            nc.sync.dma_start(bias_sb[OUT:128, :], b_src)

            n_tiles = (HC + ROWS_PER_TILE - 1) // ROWS_PER_TILE
            for p in range(BPC // 2):
                # x replicated 3x (m-shifted copies), 2 images per tile.
                x_rep = xpool.tile([128, H * W], bf16)
                for img in range(2):
                    b_idx = 2 * p + img
                    for m in range(FD):
                        p0 = img * 64 + m * 16
                        dst = x_rep[p0:p0 + C, :].rearrange(
                            "p (r j) -> p r j", j=W)[:, :, 0:W - m]
                        nc.gpsimd.dma_start(dst, x_d[b_idx, :, :, m:W])

                ostage = opool.tile([128, NPIX], f32)
                for t in range(n_tiles):
                    r0 = ROWS_PER_TILE * t
                    nrows = min(ROWS_PER_TILE, HC - r0)
                    ncols = nrows * W
                    ps = psum.tile([128, ROWS_PER_TILE * W], f32)
                    for n in range(FD):
                        off = (r0 + n) * W
                        for img in range(2):
                            pb = img * 64
                            lhsT = w_sb[pb:pb + FD * C, n * OUT:(n + 1) * OUT]
                            rhs = x_rep[pb:pb + FD * C, off:off + ncols]
                            nc.tensor.matmul(
                                ps[pb:pb + OUT, 0:ncols], lhsT, rhs,
                                start=(n == 0), stop=(n == FD - 1))

                    # relu(conv + bias): PSUM -> SBUF on ScalarE.
                    rb = rpool.tile([128, ROWS_PER_TILE * W], f32)
                    nc.scalar.activation(
                        rb[:, 0:ncols], ps[:, 0:ncols],
                        mybir.ActivationFunctionType.Relu,
                        bias=bias_sb[:, 0:1], scale=1.0)

                    # Horizontal max over column pairs: [r, 126] -> [r, 63].
                    hb = hpool.tile([128, ROWS_PER_TILE * HP], f32)
                    rb_v = rb.rearrange("p (r j2 two) -> p r j2 two",
                                        two=2, j2=W // 2)
                    hb_v = hb.rearrange("p (r j) -> p r j", j=HP)
                    nc.vector.tensor_tensor(
                        out=hb_v[:, 0:nrows, :],
                        in0=rb_v[:, 0:nrows, 0:HP, 0],
                        in1=rb_v[:, 0:nrows, 0:HP, 1],
                        op=MAX)

                    # Vertical max over row pairs, into the output stage.
                    npr = nrows // 2
                    ov = ostage[:, 2 * t * HP:(2 * t + npr) * HP].rearrange(
                        "p (pr j) -> p pr j", j=HP)
                    hb_pairs = hb.rearrange("p (pr two j) -> p pr two j",
                                            two=2, j=HP)
                    nc.vector.tensor_tensor(
                        out=ov,
                        in0=hb_pairs[:, 0:npr, 0, :],
                        in1=hb_pairs[:, 0:npr, 1, :],
                        op=MAX)

                dst = y_d[2 * p:2 * p + 2].rearrange("b o q -> (b o) q")
                nc.sync.dma_start(dst, ostage[:])

    nc.compile()
    return nc


def _get_nc():
    if "nc" not in _cache:
        _cache["nc"] = _build()
    return _cache["nc"]


def kernel(x: np.ndarray, weight: np.ndarray, bias: np.ndarray) -> np.ndarray:
    nc = _get_nc()
    x = np.ascontiguousarray(x, dtype=np.float32)
    weight = np.ascontiguousarray(weight, dtype=np.float32)
    bias = np.ascontiguousarray(bias, dtype=np.float32)
    xs = x.reshape(N_CORES, BPC, C, H, W)
    in_maps = [{"x": xs[i], "weight": weight, "bias": bias}
               for i in range(N_CORES)]
    res = run_bass_kernel_spmd(nc, in_maps, list(range(N_CORES)))
    return np.concatenate([res.results[i]["y"] for i in range(N_CORES)], axis=0)


# revision 10
# speedup vs baseline: 8932.3918x; 8932.3918x over previous
"""Trainium2 Bass kernel for ConvPool (3x3 VALID conv + bias + relu + 2x2 maxpool).

Full-input contract: kernel(x, weight, bias) -> (32, 64, 3969) float32.
Data-parallel over batch across 8 NeuronCores (4 images per core).

Per-core algorithm:
  - x is loaded into SBUF 3x (one copy per horizontal filter tap m), each
    copy shifted by m columns, at partition (m*16 + c).  Two images live at
    partition bases 0 and 64.
  - Conv = 3 PSUM-accumulating matmuls (one per vertical tap n) with K=48,
    M=64; the n-offset is a pure free-dim offset into the shifted x copy.
    The two images run in disjoint PE quadrants (rows 0-47 x cols 0-63 and
    rows 64-111 x cols 64-127) so their matmuls execute concurrently.
  - bf16 matmul inputs (cast during the SWDGE load DMA): full PE rate,
    fp32 PSUM accumulation; everything after the matmul is fp32.
  - ScalarE evacuates PSUM with relu(conv + bias) in one pass (exact:
    max-pool commutes with the monotone x -> relu(x+b)).
  - Maxpool: two DVE tensor_tensor(max) passes on SBUF (column pairs, then
    row pairs into the output stage), then one 2 MB DMA per image pair.
"""

import numpy as np

import concourse.bass as bass
import concourse.bacc as bacc
import concourse.mybir as mybir
import concourse.tile as tile
from concourse.bass_utils import run_bass_kernel_spmd

N_CORES = 8
B, C, H, W = 32, 16, 128, 128
FD, OUT, POOL = 3, 64, 2
BPC = B // N_CORES            # images per core
HC = H - FD + 1               # conv output height/width = 126
HP = HC // POOL               # pooled height/width = 63
NPIX = HP * HP                # 3969
ROWS_PER_TILE = 4             # conv rows per PSUM tile (N = 4*128 = 512)

f32 = mybir.dt.float32
bf16 = mybir.dt.bfloat16
MAX = mybir.AluOpType.max

_cache: dict = {}


def _build(loop_reps: int | None = None, mode: str = "full"):
    """Build the per-core program.  loop_reps wraps the whole body in a
    hardware For_i loop (benchmarking only: device time dominates wall).
    mode: 'full' | 'nopost' (skip ACT/DVE/out) | 'dmaonly' (x loads only)."""
    import contextlib

    nc = bacc.Bacc("TRN2", target_bir_lowering=False, debug=False)
    x_d = nc.dram_tensor("x", [BPC, C, H, W], f32, kind="ExternalInput").ap()
    w_d = nc.dram_tensor("weight", [C * FD * FD, OUT], f32,
                         kind="ExternalInput").ap()
    b_d = nc.dram_tensor("bias", [OUT], f32, kind="ExternalInput").ap()
    y_d = nc.dram_tensor("y", [BPC, OUT, NPIX], f32, kind="ExternalOutput").ap()

    with tile.TileContext(nc) as tc:
        with (
            tc.tile_pool(name="const", bufs=1) as const,
            tc.tile_pool(name="xrep", bufs=2) as xpool,
            tc.tile_pool(name="psum", bufs=4, space="PSUM") as psum,
            tc.tile_pool(name="rbuf", bufs=3) as rpool,
            tc.tile_pool(name="hbuf", bufs=3) as hpool,
            tc.tile_pool(name="ostage", bufs=2) as opool,
        ):
            # Weights: partition (img*64 + m*16 + c), free (n*64 + o).
            w_sb = const.tile([128, FD * OUT], bf16)
            w_src = w_d.rearrange("(c n m) o -> m c n o", c=C, n=FD, m=FD)
            for img in range(2):
                for m in range(FD):
                    p0 = img * 64 + m * 16
                    dst = w_sb[p0:p0 + C, :].rearrange("p (n o) -> p n o", o=OUT)
                    nc.gpsimd.dma_start(dst, w_src[m])

            # Bias: per-partition scalar, duplicated for both image halves.
            bias_sb = const.tile([128, 1], f32)
            b_src = b_d.rearrange("(o u) -> o u", u=1)
            nc.sync.dma_start(bias_sb[0:OUT, :], b_src)
            nc.sync.dma_start(bias_sb[OUT:128, :], b_src)

            n_tiles = (HC + ROWS_PER_TILE - 1) // ROWS_PER_TILE
            loop_cm = (tc.For_i(0, loop_reps, 1) if loop_reps
                       else contextlib.nullcontext())
            with loop_cm:
                _body(nc, tc, x_d, y_d, w_sb, bias_sb, n_tiles,
                      xpool, psum, rpool, hpool, opool, mode)

    nc.compile()
    return nc


def _body(nc, tc, x_d, y_d, w_sb, bias_sb, n_tiles,
          xpool, psum, rpool, hpool, opool, mode="full"):
    if True:
        for p in range(BPC // 2):
                # x replicated 3x (m-shifted copies), 2 images per tile.
                x_rep = xpool.tile([128, H * W], bf16)
                if mode == "dmaf32":
                    x_f32 = xpool.tile([128, H * W], f32, bufs=1)
                for img in range(2):
                    b_idx = 2 * p + img
                    x_flat = x_d[b_idx].rearrange("c h w -> c (h w)")
                    for m in range(FD):
                        # m-shift as a flat offset: one contiguous 64KB run
                        # per partition (row-end wraparound lands in PSUM
                        # columns the pooling never reads).
                        p0 = img * 64 + m * 16
                        if mode == "dmaf32":
                            nc.sync.dma_start(
                                x_f32[p0:p0 + C, 0:H * W - m], x_flat[:, m:])
                        else:
                            nc.gpsimd.dma_start(
                                x_rep[p0:p0 + C, 0:H * W - m], x_flat[:, m:])
                if mode == "dmaf32":
                    continue

                ostage = opool.tile([128, NPIX], f32)
                if mode == "dmaonly":
                    continue
                for t in range(n_tiles):
                    r0 = ROWS_PER_TILE * t
                    nrows = min(ROWS_PER_TILE, HC - r0)
                    ncols = nrows * W
                    ps = psum.tile([128, ROWS_PER_TILE * W], f32)
                    for n in range(FD):
                        off = (r0 + n) * W
                        for img in range(2):
                            pb = img * 64
                            lhsT = w_sb[pb:pb + FD * C, n * OUT:(n + 1) * OUT]
                            rhs = x_rep[pb:pb + FD * C, off:off + ncols]
                            nc.tensor.matmul(
                                ps[pb:pb + OUT, 0:ncols], lhsT, rhs,
                                start=(n == 0), stop=(n == FD - 1))

                    if mode == "nopost":
                        continue
                    # relu(conv + bias): PSUM -> SBUF on ScalarE.
                    rb = rpool.tile([128, ROWS_PER_TILE * W], f32)
                    nc.scalar.activation(
                        rb[:, 0:ncols], ps[:, 0:ncols],
                        mybir.ActivationFunctionType.Relu,
                        bias=bias_sb[:, 0:1], scale=1.0)

                    # Horizontal max over column pairs: [r, 126] -> [r, 63].
                    hb = hpool.tile([128, ROWS_PER_TILE * HP], f32)
                    rb_v = rb.rearrange("p (r j2 two) -> p r j2 two",
                                        two=2, j2=W // 2)
                    hb_v = hb.rearrange("p (r j) -> p r j", j=HP)
                    nc.vector.tensor_tensor(
                        out=hb_v[:, 0:nrows, :],
                        in0=rb_v[:, 0:nrows, 0:HP, 0],
                        in1=rb_v[:, 0:nrows, 0:HP, 1],
                        op=MAX)

                    # Vertical max over row pairs, into the output stage.
                    npr = nrows // 2
                    ov = ostage[:, 2 * t * HP:(2 * t + npr) * HP].rearrange(
                        "p (pr j) -> p pr j", j=HP)
                    hb_pairs = hb.rearrange("p (pr two j) -> p pr two j",
                                            two=2, j=HP)
                    nc.vector.tensor_tensor(
                        out=ov,
                        in0=hb_pairs[:, 0:npr, 0, :],
                        in1=hb_pairs[:, 0:npr, 1, :],
                        op=MAX)

                if mode == "full":
                    dst = y_d[2 * p:2 * p + 2].rearrange("b o q -> (b o) q")
                    nc.sync.dma_start(dst, ostage[:])


def _get_nc():
    if "nc" not in _cache:
        _cache["nc"] = _build()
    return _cache["nc"]


def kernel(x: np.ndarray, weight: np.ndarray, bias: np.ndarray) -> np.ndarray:
    nc = _get_nc()
    x = np.ascontiguousarray(x, dtype=np.float32)
    weight = np.ascontiguousarray(weight, dtype=np.float32)
    bias = np.ascontiguousarray(bias, dtype=np.float32)
    xs = x.reshape(N_CORES, BPC, C, H, W)
    in_maps = [{"x": xs[i], "weight": weight, "bias": bias}
               for i in range(N_CORES)]
    res = run_bass_kernel_spmd(nc, in_maps, list(range(N_CORES)))
    return np.concatenate([res.results[i]["y"] for i in range(N_CORES)], axis=0)
